# revision 30
# baseline (speedup 1.0000x reference)
"""Trainium2 Bass kernel for windowed channel-attention (nn_ChannelAttention2).

Reference computation (per batch element b):
    qkv = x @ w_qkv                    # [L, 3C], L = 36864, C = 192
    per 64-token window w:
        q, k, v = qkv[w]               # [64, C] each
        attn = softmax_d(scale * k^T v)    # [C, C] (softmax over last axis d)
        out[w] = q @ attn^T            # [64, C]
    y = out @ w_proj + b_proj

Sharding: data-parallel over batch B=8 -> one batch element per NeuronCore.
All weights replicated. No collectives.

Design (bf16 matmuls = 1 PE cycle/row vs 4 for fp32; tolerance is 2e-2 and
bf16 end-to-end measures ~1e-3):
  - x loaded fp32 on the SP HWDGE queue; PE-transposed (fp32); the
    PSUM->SBUF copy casts to bf16. Weights DMA-cast to bf16 once (gpsimd).
  - stage A is pair-batched (2 token tiles = 256 tokens): one transpose
    PSUM tile, one xT copy, one kv copy, one qT copy - halves the ACT/DVE
    instruction count for the copies.
  - k,v token-major in one matmul pair (rhs = w_qkv[:, 192:576], N=384),
    repacked window-major (64-token windows at partition 0) by one
    SBUF->SBUF DMA pair per 2 tiles, issued right after the kv copy.
  - attnT[d, c] = v^T k per window into a single 2-bank PSUM tile; ONE exp
    (ScalarE, bf16 out) covers both windows and both c-halves.
  - qT [d, tok] carries a ones column per window so the attention-output
    matmul also produces the softmax denominator D in PSUM column 64.
  - out is produced in [c, n] layout; a reciprocal pair + one broadcast
    multiply applies 1/D and casts to bf16; that tensor feeds the w_proj
    matmul directly as lhsT (no output transposes).
  - bias add fused into the final PSUM->SBUF copy; y stores are emitted one
    chunk late so their DMA generation never stalls the SP queue.

All matmul operands/outputs sit at partition 0 (tile_position (0,0)):
row/col-offset matmuls wedge the device on this stack.

reps > 1 wraps the body in a hardware loop (For_i) so one NEFF execution
repeats the computation; test.py uses it to time execution with the ~70 ms
axon dispatch overhead amortized away.
"""

import os

import numpy as np

C = 192
L = 36864
N_CORES = 8
WIN = 64
TOK_TILE = 128  # 2 windows
CH = int(os.environ.get("K_CH", "8"))  # token tiles per chunk
SCALE = float((C // 8) ** -0.5)

_CACHE = {}


def _build(length=L, n_cores=N_CORES, reps=1):
    import concourse.bass as bass
    import concourse.mybir as mybir
    import concourse.tile as tile
    from concourse import bacc
    from concourse.masks import make_identity
    from contextlib import nullcontext

    f32 = mybir.dt.float32
    bf16 = mybir.dt.bfloat16
    AF = mybir.ActivationFunctionType

    n_tiles = length // TOK_TILE
    n_chunks = n_tiles // CH
    assert n_chunks * CH == n_tiles and CH % 2 == 0

    nc = bacc.Bacc("TRN2", target_bir_lowering=False, debug=False,
                   num_devices=n_cores)
    x_d = nc.declare_dram_parameter("x", [length, C], f32, isOutput=False)
    wqkv_d = nc.declare_dram_parameter("w_qkv", [C, 3 * C], f32, isOutput=False)
    wp_d = nc.declare_dram_parameter("w_proj", [C, C], f32, isOutput=False)
    bp_d = nc.declare_dram_parameter("b_proj", [C], f32, isOutput=False)
    y_d = nc.declare_dram_parameter("y", [length, C], f32, isOutput=True)

    with tile.TileContext(nc) as tc:
        with (
            tc.tile_pool(name="singles", bufs=1) as singles,
            tc.tile_pool(name="ck", bufs=3) as ck,
            tc.tile_pool(name="sb", bufs=4) as sb,
            tc.tile_pool(name="ps_small", bufs=2, space="PSUM") as ps_small,
            tc.tile_pool(name="ps_kv", bufs=1, space="PSUM") as ps_kv,
            tc.tile_pool(name="ps_attn", bufs=1, space="PSUM") as ps_attn,
            tc.tile_pool(name="ps_oy", bufs=2, space="PSUM") as ps_oy,
        ):
            # ---- constants / weights (loaded once, cast to bf16 in DMA) ----
            ident = singles.tile([128, 128], f32)
            make_identity(nc, ident)
            wqkv_bh = singles.tile([128, 3 * C], bf16)
            nc.gpsimd.dma_start(out=wqkv_bh, in_=wqkv_d[0:128, :])
            wqkv_bl = singles.tile([64, 3 * C], bf16)
            nc.gpsimd.dma_start(out=wqkv_bl, in_=wqkv_d[128:192, :])
            wp_bh = singles.tile([128, C], bf16)
            nc.gpsimd.dma_start(out=wp_bh, in_=wp_d[0:128, :])
            wp_bl = singles.tile([64, C], bf16)
            nc.gpsimd.dma_start(out=wp_bl, in_=wp_d[128:192, :])
            b_sb = singles.tile([128, C], f32)
            nc.gpsimd.dma_start(
                out=b_sb,
                in_=bass.AP(tensor=bp_d, offset=0, ap=[[0, 128], [1, C]]))

            rep_ctx = tc.For_i(0, reps, 1, name="rep") if reps > 1 \
                else nullcontext()
            with rep_ctx:
                pending_store = []  # chunk-delayed y stores

                def stage_a_pair(t0, x_sb, kv_sb, kv2_sb):
                    # x pair -> xT (PE transpose fp32; copy casts to bf16)
                    xq_ps = ps_small.tile([128, 2, 256], f32, tag="small")
                    for i in (0, 1):
                        nc.tensor.transpose(xq_ps[:, i, 0:128],
                                            x_sb[:, t0 + i, 0:128], ident)
                        nc.tensor.transpose(xq_ps[0:64, i, 128:256],
                                            x_sb[:, t0 + i, 128:192], ident)
                    xT2_sb = sb.tile([128, 2, 256], bf16, tag="xT")
                    nc.scalar.copy(xT2_sb, xq_ps)

                    # k,v token-major; each tile's kv padded to one PSUM bank
                    kv_ps = ps_kv.tile([128, 2, 512], f32, tag="kv")
                    for i in (0, 1):
                        nc.tensor.matmul(kv_ps[:, i, 0:2 * C],
                                         xT2_sb[:, i, 0:128],
                                         wqkv_bh[:, C:3 * C],
                                         start=True, stop=False)
                        nc.tensor.matmul(kv_ps[:, i, 0:2 * C],
                                         xT2_sb[0:64, i, 128:256],
                                         wqkv_bl[:, C:3 * C],
                                         start=False, stop=True)
                    nc.vector.tensor_copy(kv_sb[:, t0:t0 + 2, :],
                                          kv_ps[:, :, 0:2 * C])
                    # window 0 already sits at partitions 0:64 of kv_sb, so
                    # only window 1 (partitions 64:128) needs the repack DMA
                    sl = slice(t0, t0 + 2)
                    nc.sync.dma_start(out=kv2_sb[:, sl, :],
                                      in_=kv_sb[64:128, sl, :])

                    # qT channel-major [j, tok] + ones column per window
                    qT_ps = ps_small.tile([128, 2, 256], f32, tag="small")
                    for i in (0, 1):
                        xT_hi = xT2_sb[:, i, 0:128]
                        xT_lo = xT2_sb[0:64, i, 128:256]
                        nc.tensor.matmul(qT_ps[:, i, 0:128],
                                         wqkv_bh[:, 0:128], xT_hi,
                                         start=True, stop=False)
                        nc.tensor.matmul(qT_ps[:, i, 0:128],
                                         wqkv_bl[:, 0:128], xT_lo,
                                         start=False, stop=True)
                        nc.tensor.matmul(qT_ps[0:64, i, 128:256],
                                         wqkv_bh[:, 128:192], xT_hi,
                                         start=True, stop=False)
                        nc.tensor.matmul(qT_ps[0:64, i, 128:256],
                                         wqkv_bl[:, 128:192], xT_lo,
                                         start=False, stop=True)
                    # [:, i, g, 0:64] = tokens, [:, i, g, 64] = 1.0;
                    # g in {0,1}: d_hi windows, {2,3}: d_lo windows
                    qT2_sb = sb.tile([128, 2, 4, WIN + 1], bf16, tag="qT")
                    nc.scalar.copy(
                        qT2_sb[:, :, :, 0:WIN],
                        qT_ps.rearrange("p i (g n) -> p i g n", g=4))
                    nc.gpsimd.memset(qT2_sb[:, :, :, WIN:WIN + 1], 1.0)
                    return qT2_sb

                def stage_b(t, qT_sb, kv2_sb, fin_sb):
                    if os.environ.get("K_ABLATE") == "nob":
                        nc.vector.memset(fin_sb[:, t, :], 0.5)
                        return
                    # attnT[d, c] = sum_n v[n, d] k[n, c] per window, into a
                    # 2-bank PSUM tile (window w at float offset 512*w):
                    # hi = [:, w, 0:192], lo = [0:64, w, 192:384]
                    at_ps = ps_attn.tile([128, 2, 512], f32, tag="attn")
                    for w, kvw in ((0, kv_sb[0:64, t, :]),
                                   (1, kv2_sb[:, t, :])):
                        k_ap = kvw[:, 0:C]
                        nc.tensor.matmul(at_ps[:, w, 0:C],
                                         kvw[:, C:C + 128], k_ap,
                                         start=True, stop=True)
                        nc.tensor.matmul(at_ps[0:64, w, C:2 * C],
                                         kvw[:, C + 128:2 * C], k_ap,
                                         start=True, stop=True)

                    # one exp for both windows and both c-halves (rows 64:128
                    # of each lo half are junk and never read)
                    E_sb = sb.tile([128, 2, 2 * C], bf16, tag="E")
                    nc.scalar.activation(E_sb, at_ps[:, :, 0:2 * C], AF.Exp,
                                         scale=SCALE)
                    Ev = E_sb.rearrange("p w (h c) -> p w h c", h=2)
                    Eh_sb = Ev[:, :, 0, :]
                    El_sb = Ev[:, :, 1, :]

                    # out_ps[c, 0:64] = sum_d E[d, c] q[n, d]; [c, 64] = D[c]
                    # groups 0,1 = c_hi windows; 2,3 = c_lo windows.
                    # y (proj output) shares the same PSUM bank pair.
                    oy_ps = ps_oy.tile([128, 452], f32, tag="oy")
                    out_ps = oy_ps[:, 0:260].rearrange("p (g n) -> p g n", g=4)
                    y_ps = oy_ps[:, 260:452]
                    for w in range(2):
                        nc.tensor.matmul(out_ps[:, w, :], Eh_sb[:, w, 0:128],
                                         qT_sb[:, w, :], start=True, stop=False)
                        nc.tensor.matmul(out_ps[:, w, :], El_sb[0:64, w, 0:128],
                                         qT_sb[0:64, 2 + w, :],
                                         start=False, stop=True)
                        nc.tensor.matmul(out_ps[0:64, 2 + w, :],
                                         Eh_sb[:, w, 128:192],
                                         qT_sb[:, w, :], start=True, stop=False)
                        nc.tensor.matmul(out_ps[0:64, 2 + w, :],
                                         El_sb[0:64, w, 128:192],
                                         qT_sb[0:64, 2 + w, :],
                                         start=False, stop=True)

                    # 1/D, then one broadcast divide+cast for all 4 groups
                    # (rows 64:128 of the c_lo groups are junk; never read)
                    rD = sb.tile([128, 4, 1], f32, tag="rD")
                    nc.vector.reciprocal_approx_fast(
                        out=rD, in_=out_ps[:, :, WIN:WIN + 1])
                    out_b = sb.tile([128, 4, WIN], bf16, tag="ob")
                    nc.vector.tensor_mul(out_b, out_ps[:, :, 0:WIN],
                                         rD.broadcast_to([128, 4, WIN]))

                    # proj: lhsT = out[c, n] directly; bias in the final copy
                    nc.tensor.matmul(y_ps, out_b[:, 0:2, :], wp_bh,
                                     start=True, stop=False)
                    nc.tensor.matmul(y_ps, out_b[0:64, 2:4, :], wp_bl,
                                     start=False, stop=True)
                    nc.vector.tensor_add(fin_sb[:, t, :], y_ps, b_sb)

                for ci in range(n_chunks):
                    row0 = ci * CH * TOK_TILE
                    x_chunk = x_d[row0:row0 + CH * TOK_TILE, :].rearrange(
                        "(t p) c -> p t c", p=TOK_TILE)
                    y_chunk = y_d[row0:row0 + CH * TOK_TILE, :].rearrange(
                        "(t p) c -> p t c", p=TOK_TILE)

                    x_sb = ck.tile([128, CH, C], f32, tag="x_sb")
                    nc.sync.dma_start(out=x_sb, in_=x_chunk)
                    kv_sb = ck.tile([128, CH, 2 * C], bf16, tag="kv_sb")
                    kv2_sb = ck.tile([64, CH, 2 * C], bf16, tag="kv2_sb")
                    if os.environ.get("K_ABLATE") == "norepack":
                        nc.gpsimd.memset(kv2_sb, 0.125)
                    fin_sb = ck.tile([128, CH, C], f32, tag="fin_sb")

                    RB = int(os.environ.get("K_RB", "1"))
                    for g in range(0, CH // 2, RB):
                        qts = []
                        for tp in range(g, g + RB):
                            t0 = 2 * tp
                            qts.append(
                                (t0, stage_a_pair(t0, x_sb, kv_sb, kv2_sb)))
                        for t0, qT2_sb in qts:
                            stage_b(t0, qT2_sb[:, 0, :, :], kv2_sb, fin_sb)
                            stage_b(t0 + 1, qT2_sb[:, 1, :, :], kv2_sb,
                                    fin_sb)

                    # emit the y store one chunk late: by then its data is
                    # complete, so the DMA generation never blocks the SP
                    # queue ahead of the next chunk's repacks
                    pending_store.append((y_chunk, fin_sb))
                    if len(pending_store) > 1:
                        yc, fs = pending_store.pop(0)
                        nc.sync.dma_start(out=yc, in_=fs)
                for yc, fs in pending_store:
                    nc.sync.dma_start(out=yc, in_=fs)

    nc.compile()
    return nc


def _get_nc(length=L, n_cores=N_CORES, reps=1):
    key = (length, n_cores, reps, CH)
    if key not in _CACHE:
        _CACHE[key] = _build(length, n_cores, reps)
    return _CACHE[key]


def kernel(x, w_qkv, w_proj, b_proj, H=None, W=None, **_unused):
    from concourse.bass_utils import run_bass_kernel_spmd

    x = np.asarray(x, dtype=np.float32)
    w_qkv = np.asarray(w_qkv, dtype=np.float32)
    w_proj = np.asarray(w_proj, dtype=np.float32)
    b_proj = np.asarray(b_proj, dtype=np.float32)
    B, length, c = x.shape
    assert B == N_CORES and c == C

    nc = _get_nc(length, N_CORES)
    in_maps = [
        {"x": np.ascontiguousarray(x[b]), "w_qkv": w_qkv, "w_proj": w_proj,
         "b_proj": b_proj}
        for b in range(B)
    ]
    res = run_bass_kernel_spmd(nc, in_maps, list(range(N_CORES)))
    return np.stack([res.results[b]["y"] for b in range(B)], axis=0)


if __name__ == "__main__":
    # mini smoke test: one chunk worth of tokens per core
    length = int(os.environ.get("K_LEN", CH * TOK_TILE))
    rng = np.random.default_rng(0)
    x = rng.standard_normal((N_CORES, length, C), dtype=np.float32)
    w_qkv = (rng.standard_normal((C, 3 * C)) * 0.02).astype(np.float32)
    w_proj = (rng.standard_normal((C, C)) * 0.02).astype(np.float32)
    b_proj = (rng.standard_normal((C,)) * 0.02).astype(np.float32)

    def ref(x):
        qkv = x @ w_qkv  # [B, L, 3C]
        B_, L_, _ = x.shape
        qkv = qkv.reshape(B_, L_ // 64, 64, 3, C)
        q, k, v = qkv[..., 0, :], qkv[..., 1, :] * SCALE, qkv[..., 2, :]
        attn = np.einsum('bwnc,bwnd->bwcd', k, v)
        attn = np.exp(attn - attn.max(-1, keepdims=True))
        attn = attn / attn.sum(-1, keepdims=True)
        out = np.einsum('bwcd,bwnd->bwnc', attn, q).reshape(B_, L_, C)
        return out @ w_proj + b_proj

    expected = ref(x)
    got = kernel(x, w_qkv, w_proj, b_proj)
    err = np.abs(got - expected).max()
    rel = np.linalg.norm(got - expected) / np.linalg.norm(expected)
    print(f"mini test: max abs err {err:.3e}  rel fro {rel:.3e}")


# revision 31
# speedup vs baseline: 1.0068x; 1.0068x over previous
"""Trainium2 Bass kernel for windowed channel-attention (nn_ChannelAttention2).

Reference computation (per batch element b):
    qkv = x @ w_qkv                    # [L, 3C], L = 36864, C = 192
    per 64-token window w:
        q, k, v = qkv[w]               # [64, C] each
        attn = softmax_d(scale * k^T v)    # [C, C] (softmax over last axis d)
        out[w] = q @ attn^T            # [64, C]
    y = out @ w_proj + b_proj

Sharding: data-parallel over batch B=8 -> one batch element per NeuronCore.
All weights replicated. No collectives.

Design (bf16 matmuls = 1 PE cycle/row vs 4 for fp32; tolerance is 2e-2 and
bf16 end-to-end measures ~1e-3):
  - x loaded fp32 on the SP HWDGE queue; PE-transposed (fp32); the
    PSUM->SBUF copy casts to bf16. Weights DMA-cast to bf16 once (gpsimd).
  - stage A is pair-batched (2 token tiles = 256 tokens): one transpose
    PSUM tile, one xT copy, one kv copy, one qT copy - halves the ACT/DVE
    instruction count for the copies.
  - k,v token-major in one matmul pair (rhs = w_qkv[:, 192:576], N=384),
    repacked window-major (64-token windows at partition 0) by one
    SBUF->SBUF DMA pair per 2 tiles, issued right after the kv copy.
  - attnT[d, c] = v^T k per window into a single 2-bank PSUM tile; ONE exp
    (ScalarE, bf16 out) covers both windows and both c-halves.
  - qT [d, tok] carries a ones column per window so the attention-output
    matmul also produces the softmax denominator D in PSUM column 64.
  - out is produced in [c, n] layout; a reciprocal pair + one broadcast
    multiply applies 1/D and casts to bf16; that tensor feeds the w_proj
    matmul directly as lhsT (no output transposes).
  - bias add fused into the final PSUM->SBUF copy; y stores are emitted one
    chunk late so their DMA generation never stalls the SP queue.

All matmul operands/outputs sit at partition 0 (tile_position (0,0)):
row/col-offset matmuls wedge the device on this stack.

reps > 1 wraps the body in a hardware loop (For_i) so one NEFF execution
repeats the computation; test.py uses it to time execution with the ~70 ms
axon dispatch overhead amortized away.
"""

import os

import numpy as np

C = 192
L = 36864
N_CORES = 8
WIN = 64
TOK_TILE = 128  # 2 windows
CH = int(os.environ.get("K_CH", "8"))  # token tiles per chunk
SCALE = float((C // 8) ** -0.5)

_CACHE = {}


def _build(length=L, n_cores=N_CORES, reps=1):
    import concourse.bass as bass
    import concourse.mybir as mybir
    import concourse.tile as tile
    from concourse import bacc
    from concourse.masks import make_identity
    from contextlib import nullcontext

    f32 = mybir.dt.float32
    bf16 = mybir.dt.bfloat16
    AF = mybir.ActivationFunctionType

    n_tiles = length // TOK_TILE
    n_chunks = n_tiles // CH
    assert n_chunks * CH == n_tiles and CH % 2 == 0

    nc = bacc.Bacc("TRN2", target_bir_lowering=False, debug=False,
                   num_devices=n_cores)
    x_d = nc.declare_dram_parameter("x", [length, C], f32, isOutput=False)
    wqkv_d = nc.declare_dram_parameter("w_qkv", [C, 3 * C], f32, isOutput=False)
    wp_d = nc.declare_dram_parameter("w_proj", [C, C], f32, isOutput=False)
    bp_d = nc.declare_dram_parameter("b_proj", [C], f32, isOutput=False)
    y_d = nc.declare_dram_parameter("y", [length, C], f32, isOutput=True)

    with tile.TileContext(nc) as tc:
        with (
            tc.tile_pool(name="singles", bufs=1) as singles,
            tc.tile_pool(name="ck", bufs=4) as ck,
            tc.tile_pool(name="sb", bufs=6) as sb,
            tc.tile_pool(name="ps_small", bufs=2, space="PSUM") as ps_small,
            tc.tile_pool(name="ps_kv", bufs=1, space="PSUM") as ps_kv,
            tc.tile_pool(name="ps_attn", bufs=1, space="PSUM") as ps_attn,
            tc.tile_pool(name="ps_oy", bufs=2, space="PSUM") as ps_oy,
        ):
            # ---- constants / weights (loaded once, cast to bf16 in DMA) ----
            ident = singles.tile([128, 128], f32)
            make_identity(nc, ident)
            wqkv_bh = singles.tile([128, 3 * C], bf16)
            nc.gpsimd.dma_start(out=wqkv_bh, in_=wqkv_d[0:128, :])
            wqkv_bl = singles.tile([64, 3 * C], bf16)
            nc.gpsimd.dma_start(out=wqkv_bl, in_=wqkv_d[128:192, :])
            wp_bh = singles.tile([128, C], bf16)
            nc.gpsimd.dma_start(out=wp_bh, in_=wp_d[0:128, :])
            wp_bl = singles.tile([64, C], bf16)
            nc.gpsimd.dma_start(out=wp_bl, in_=wp_d[128:192, :])
            b_sb = singles.tile([128, C], f32)
            nc.gpsimd.dma_start(
                out=b_sb,
                in_=bass.AP(tensor=bp_d, offset=0, ap=[[0, 128], [1, C]]))

            rep_ctx = tc.For_i(0, reps, 1, name="rep") if reps > 1 \
                else nullcontext()
            with rep_ctx:
                pending_store = []  # chunk-delayed y stores

                def stage_a_pair(t0, x_sb, kv_sb, kv2_sb):
                    # x pair -> xT (PE transpose fp32; copy casts to bf16)
                    xq_ps = ps_small.tile([128, 2, 256], f32, tag="small")
                    for i in (0, 1):
                        nc.tensor.transpose(xq_ps[:, i, 0:128],
                                            x_sb[:, t0 + i, 0:128], ident)
                        nc.tensor.transpose(xq_ps[0:64, i, 128:256],
                                            x_sb[:, t0 + i, 128:192], ident)
                    xT2_sb = sb.tile([128, 2, 256], bf16, tag="xT")
                    nc.scalar.copy(xT2_sb, xq_ps)

                    # k,v token-major; each tile's kv padded to one PSUM bank
                    kv_ps = ps_kv.tile([128, 2, 512], f32, tag="kv")
                    for i in (0, 1):
                        nc.tensor.matmul(kv_ps[:, i, 0:2 * C],
                                         xT2_sb[:, i, 0:128],
                                         wqkv_bh[:, C:3 * C],
                                         start=True, stop=False)
                        nc.tensor.matmul(kv_ps[:, i, 0:2 * C],
                                         xT2_sb[0:64, i, 128:256],
                                         wqkv_bl[:, C:3 * C],
                                         start=False, stop=True)
                    nc.vector.tensor_copy(kv_sb[:, t0:t0 + 2, :],
                                          kv_ps[:, :, 0:2 * C])
                    # window 0 already sits at partitions 0:64 of kv_sb, so
                    # only window 1 (partitions 64:128) needs the repack DMA
                    sl = slice(t0, t0 + 2)
                    nc.sync.dma_start(out=kv2_sb[:, sl, :],
                                      in_=kv_sb[64:128, sl, :])

                    # qT channel-major [j, tok] + ones column per window
                    qT_ps = ps_small.tile([128, 2, 256], f32, tag="small")
                    for i in (0, 1):
                        xT_hi = xT2_sb[:, i, 0:128]
                        xT_lo = xT2_sb[0:64, i, 128:256]
                        nc.tensor.matmul(qT_ps[:, i, 0:128],
                                         wqkv_bh[:, 0:128], xT_hi,
                                         start=True, stop=False)
                        nc.tensor.matmul(qT_ps[:, i, 0:128],
                                         wqkv_bl[:, 0:128], xT_lo,
                                         start=False, stop=True)
                        nc.tensor.matmul(qT_ps[0:64, i, 128:256],
                                         wqkv_bh[:, 128:192], xT_hi,
                                         start=True, stop=False)
                        nc.tensor.matmul(qT_ps[0:64, i, 128:256],
                                         wqkv_bl[:, 128:192], xT_lo,
                                         start=False, stop=True)
                    # [:, i, g, 0:64] = tokens, [:, i, g, 64] = 1.0;
                    # g in {0,1}: d_hi windows, {2,3}: d_lo windows
                    qT2_sb = sb.tile([128, 2, 4, WIN + 1], bf16, tag="qT")
                    nc.scalar.copy(
                        qT2_sb[:, :, :, 0:WIN],
                        qT_ps.rearrange("p i (g n) -> p i g n", g=4))
                    nc.gpsimd.memset(qT2_sb[:, :, :, WIN:WIN + 1], 1.0)
                    return qT2_sb

                def stage_b(t, qT_sb, kv2_sb, fin_sb):
                    if os.environ.get("K_ABLATE") == "nob":
                        nc.vector.memset(fin_sb[:, t, :], 0.5)
                        return
                    # attnT[d, c] = sum_n v[n, d] k[n, c] per window, into a
                    # 2-bank PSUM tile (window w at float offset 512*w):
                    # hi = [:, w, 0:192], lo = [0:64, w, 192:384]
                    at_ps = ps_attn.tile([128, 2, 512], f32, tag="attn")
                    for w, kvw in ((0, kv_sb[0:64, t, :]),
                                   (1, kv2_sb[:, t, :])):
                        k_ap = kvw[:, 0:C]
                        nc.tensor.matmul(at_ps[:, w, 0:C],
                                         kvw[:, C:C + 128], k_ap,
                                         start=True, stop=True)
                        nc.tensor.matmul(at_ps[0:64, w, C:2 * C],
                                         kvw[:, C + 128:2 * C], k_ap,
                                         start=True, stop=True)

                    # one exp for both windows and both c-halves (rows 64:128
                    # of each lo half are junk and never read)
                    E_sb = sb.tile([128, 2, 2 * C], bf16, tag="E")
                    nc.scalar.activation(E_sb, at_ps[:, :, 0:2 * C], AF.Exp,
                                         scale=SCALE)
                    Ev = E_sb.rearrange("p w (h c) -> p w h c", h=2)
                    Eh_sb = Ev[:, :, 0, :]
                    El_sb = Ev[:, :, 1, :]

                    # out_ps[c, 0:64] = sum_d E[d, c] q[n, d]; [c, 64] = D[c]
                    # groups 0,1 = c_hi windows; 2,3 = c_lo windows.
                    # y (proj output) shares the same PSUM bank pair.
                    oy_ps = ps_oy.tile([128, 452], f32, tag="oy")
                    out_ps = oy_ps[:, 0:260].rearrange("p (g n) -> p g n", g=4)
                    y_ps = oy_ps[:, 260:452]
                    for w in range(2):
                        nc.tensor.matmul(out_ps[:, w, :], Eh_sb[:, w, 0:128],
                                         qT_sb[:, w, :], start=True, stop=False)
                        nc.tensor.matmul(out_ps[:, w, :], El_sb[0:64, w, 0:128],
                                         qT_sb[0:64, 2 + w, :],
                                         start=False, stop=True)
                        nc.tensor.matmul(out_ps[0:64, 2 + w, :],
                                         Eh_sb[:, w, 128:192],
                                         qT_sb[:, w, :], start=True, stop=False)
                        nc.tensor.matmul(out_ps[0:64, 2 + w, :],
                                         El_sb[0:64, w, 128:192],
                                         qT_sb[0:64, 2 + w, :],
                                         start=False, stop=True)

                    # 1/D, then one broadcast divide+cast for all 4 groups
                    # (rows 64:128 of the c_lo groups are junk; never read)
                    rD = sb.tile([128, 4, 1], f32, tag="rD")
                    nc.vector.reciprocal_approx_fast(
                        out=rD, in_=out_ps[:, :, WIN:WIN + 1])
                    out_b = sb.tile([128, 4, WIN], bf16, tag="ob")
                    nc.vector.tensor_mul(out_b, out_ps[:, :, 0:WIN],
                                         rD.broadcast_to([128, 4, WIN]))

                    # proj: lhsT = out[c, n] directly; bias in the final copy
                    nc.tensor.matmul(y_ps, out_b[:, 0:2, :], wp_bh,
                                     start=True, stop=False)
                    nc.tensor.matmul(y_ps, out_b[0:64, 2:4, :], wp_bl,
                                     start=False, stop=True)
                    nc.vector.tensor_add(fin_sb[:, t, :], y_ps, b_sb)

                for ci in range(n_chunks):
                    row0 = ci * CH * TOK_TILE
                    x_chunk = x_d[row0:row0 + CH * TOK_TILE, :].rearrange(
                        "(t p) c -> p t c", p=TOK_TILE)
                    y_chunk = y_d[row0:row0 + CH * TOK_TILE, :].rearrange(
                        "(t p) c -> p t c", p=TOK_TILE)

                    x_sb = ck.tile([128, CH, C], f32, tag="x_sb")
                    nc.sync.dma_start(out=x_sb, in_=x_chunk)
                    kv_sb = ck.tile([128, CH, 2 * C], bf16, tag="kv_sb")
                    kv2_sb = ck.tile([64, CH, 2 * C], bf16, tag="kv2_sb")
                    if os.environ.get("K_ABLATE") == "norepack":
                        nc.gpsimd.memset(kv2_sb, 0.125)
                    fin_sb = ck.tile([128, CH, C], f32, tag="fin_sb")

                    RB = int(os.environ.get("K_RB", "1"))
                    for g in range(0, CH // 2, RB):
                        qts = []
                        for tp in range(g, g + RB):
                            t0 = 2 * tp
                            qts.append(
                                (t0, stage_a_pair(t0, x_sb, kv_sb, kv2_sb)))
                        for t0, qT2_sb in qts:
                            stage_b(t0, qT2_sb[:, 0, :, :], kv2_sb, fin_sb)
                            stage_b(t0 + 1, qT2_sb[:, 1, :, :], kv2_sb,
                                    fin_sb)

                    # emit the y store one chunk late: by then its data is
                    # complete, so the DMA generation never blocks the SP
                    # queue ahead of the next chunk's repacks
                    pending_store.append((y_chunk, fin_sb))
                    if len(pending_store) > 1:
                        yc, fs = pending_store.pop(0)
                        nc.sync.dma_start(out=yc, in_=fs)
                for yc, fs in pending_store:
                    nc.sync.dma_start(out=yc, in_=fs)

    nc.compile()
    return nc


def _get_nc(length=L, n_cores=N_CORES, reps=1):
    key = (length, n_cores, reps, CH)
    if key not in _CACHE:
        _CACHE[key] = _build(length, n_cores, reps)
    return _CACHE[key]


def kernel(x, w_qkv, w_proj, b_proj, H=None, W=None, **_unused):
    from concourse.bass_utils import run_bass_kernel_spmd

    x = np.asarray(x, dtype=np.float32)
    w_qkv = np.asarray(w_qkv, dtype=np.float32)
    w_proj = np.asarray(w_proj, dtype=np.float32)
    b_proj = np.asarray(b_proj, dtype=np.float32)
    B, length, c = x.shape
    assert B == N_CORES and c == C

    nc = _get_nc(length, N_CORES)
    in_maps = [
        {"x": np.ascontiguousarray(x[b]), "w_qkv": w_qkv, "w_proj": w_proj,
         "b_proj": b_proj}
        for b in range(B)
    ]
    res = run_bass_kernel_spmd(nc, in_maps, list(range(N_CORES)))
    return np.stack([res.results[b]["y"] for b in range(B)], axis=0)


if __name__ == "__main__":
    # mini smoke test: one chunk worth of tokens per core
    length = int(os.environ.get("K_LEN", CH * TOK_TILE))
    rng = np.random.default_rng(0)
    x = rng.standard_normal((N_CORES, length, C), dtype=np.float32)
    w_qkv = (rng.standard_normal((C, 3 * C)) * 0.02).astype(np.float32)
    w_proj = (rng.standard_normal((C, C)) * 0.02).astype(np.float32)
    b_proj = (rng.standard_normal((C,)) * 0.02).astype(np.float32)

    def ref(x):
        qkv = x @ w_qkv  # [B, L, 3C]
        B_, L_, _ = x.shape
        qkv = qkv.reshape(B_, L_ // 64, 64, 3, C)
        q, k, v = qkv[..., 0, :], qkv[..., 1, :] * SCALE, qkv[..., 2, :]
        attn = np.einsum('bwnc,bwnd->bwcd', k, v)
        attn = np.exp(attn - attn.max(-1, keepdims=True))
        attn = attn / attn.sum(-1, keepdims=True)
        out = np.einsum('bwcd,bwnd->bwnc', attn, q).reshape(B_, L_, C)
        return out @ w_proj + b_proj

    expected = ref(x)
    got = kernel(x, w_qkv, w_proj, b_proj)
    err = np.abs(got - expected).max()
    rel = np.linalg.norm(got - expected) / np.linalg.norm(expected)
    print(f"mini test: max abs err {err:.3e}  rel fro {rel:.3e}")


# revision 33
# speedup vs baseline: 1.0313x; 1.0243x over previous
"""Trainium2 Bass kernel for windowed channel-attention (nn_ChannelAttention2).

Reference computation (per batch element b):
    qkv = x @ w_qkv                    # [L, 3C], L = 36864, C = 192
    per 64-token window w:
        q, k, v = qkv[w]               # [64, C] each
        attn = softmax_d(scale * k^T v)    # [C, C] (softmax over last axis d)
        out[w] = q @ attn^T            # [64, C]
    y = out @ w_proj + b_proj

Sharding: data-parallel over batch B=8 -> one batch element per NeuronCore.
All weights replicated. No collectives.

Design (bf16 matmuls = 1 PE cycle/row vs 4 for fp32; tolerance is 2e-2 and
bf16 end-to-end measures ~1e-3):
  - x loaded fp32 on the SP HWDGE queue; PE-transposed (fp32); the
    PSUM->SBUF copy casts to bf16. Weights DMA-cast to bf16 once (gpsimd).
  - stage A is pair-batched (2 token tiles = 256 tokens): one transpose
    PSUM tile, one xT copy, one kv copy, one qT copy - halves the ACT/DVE
    instruction count for the copies.
  - k,v token-major in one matmul pair (rhs = w_qkv[:, 192:576], N=384),
    repacked window-major (64-token windows at partition 0) by one
    SBUF->SBUF DMA pair per 2 tiles, issued right after the kv copy.
  - attnT[d, c] = v^T k per window into a single 2-bank PSUM tile; ONE exp
    (ScalarE, bf16 out) covers both windows and both c-halves.
  - qT [d, tok] carries a ones column per window so the attention-output
    matmul also produces the softmax denominator D in PSUM column 64.
  - out is produced in [c, n] layout; a reciprocal pair + one broadcast
    multiply applies 1/D and casts to bf16; that tensor feeds the w_proj
    matmul directly as lhsT (no output transposes).
  - bias add fused into the final PSUM->SBUF copy; y stores are emitted one
    chunk late so their DMA generation never stalls the SP queue.

All matmul operands/outputs sit at partition 0 (tile_position (0,0)):
row/col-offset matmuls wedge the device on this stack.

reps > 1 wraps the body in a hardware loop (For_i) so one NEFF execution
repeats the computation; test.py uses it to time execution with the ~70 ms
axon dispatch overhead amortized away.
"""

import os

import numpy as np

C = 192
L = 36864
N_CORES = 8
WIN = 64
TOK_TILE = 128  # 2 windows
CH = int(os.environ.get("K_CH", "8"))  # token tiles per chunk
SCALE = float((C // 8) ** -0.5)

_CACHE = {}


def _build(length=L, n_cores=N_CORES, reps=1):
    import concourse.bass as bass
    import concourse.mybir as mybir
    import concourse.tile as tile
    from concourse import bacc
    from concourse.masks import make_identity
    from contextlib import nullcontext

    f32 = mybir.dt.float32
    bf16 = mybir.dt.bfloat16
    AF = mybir.ActivationFunctionType

    n_tiles = length // TOK_TILE
    n_chunks = n_tiles // CH
    assert n_chunks * CH == n_tiles and CH % 2 == 0

    nc = bacc.Bacc("TRN2", target_bir_lowering=False, debug=False,
                   num_devices=n_cores)
    x_d = nc.declare_dram_parameter("x", [length, C], f32, isOutput=False)
    wqkv_d = nc.declare_dram_parameter("w_qkv", [C, 3 * C], f32, isOutput=False)
    wp_d = nc.declare_dram_parameter("w_proj", [C, C], f32, isOutput=False)
    bp_d = nc.declare_dram_parameter("b_proj", [C], f32, isOutput=False)
    y_d = nc.declare_dram_parameter("y", [length, C], f32, isOutput=True)

    with tile.TileContext(nc) as tc:
        with (
            tc.tile_pool(name="singles", bufs=1) as singles,
            tc.tile_pool(name="ck", bufs=3) as ck,
            tc.tile_pool(name="sb", bufs=4) as sb,
            tc.tile_pool(name="ps_small", bufs=2, space="PSUM") as ps_small,
            tc.tile_pool(name="ps_kv", bufs=1, space="PSUM") as ps_kv,
            tc.tile_pool(name="ps_attn", bufs=1, space="PSUM") as ps_attn,
            tc.tile_pool(name="ps_oy", bufs=1, space="PSUM") as ps_oy,
        ):
            # ---- constants / weights (loaded once, cast to bf16 in DMA) ----
            ident = singles.tile([128, 128], f32)
            make_identity(nc, ident)
            wqkv_bh = singles.tile([128, 3 * C], bf16)
            nc.gpsimd.dma_start(out=wqkv_bh, in_=wqkv_d[0:128, :])
            wqkv_bl = singles.tile([64, 3 * C], bf16)
            nc.gpsimd.dma_start(out=wqkv_bl, in_=wqkv_d[128:192, :])
            wp_bh = singles.tile([128, C], bf16)
            nc.gpsimd.dma_start(out=wp_bh, in_=wp_d[0:128, :])
            wp_bl = singles.tile([64, C], bf16)
            nc.gpsimd.dma_start(out=wp_bl, in_=wp_d[128:192, :])
            b_sb = singles.tile([128, C], f32)
            nc.gpsimd.dma_start(
                out=b_sb,
                in_=bass.AP(tensor=bp_d, offset=0, ap=[[0, 128], [1, C]]))

            rep_ctx = tc.For_i(0, reps, 1, name="rep") if reps > 1 \
                else nullcontext()
            with rep_ctx:
                pending_store = []  # chunk-delayed y stores

                def stage_a_pair(t0, x_sb, kv_sb, kv2_sb):
                    # x pair -> xT (PE transpose fp32; copy casts to bf16)
                    xq_ps = ps_small.tile([128, 2, 256], f32, tag="small")
                    for i in (0, 1):
                        nc.tensor.transpose(xq_ps[:, i, 0:128],
                                            x_sb[:, t0 + i, 0:128], ident)
                        nc.tensor.transpose(xq_ps[0:64, i, 128:256],
                                            x_sb[:, t0 + i, 128:192], ident)
                    xT2_sb = sb.tile([128, 2, 256], bf16, tag="xT")
                    nc.scalar.copy(xT2_sb, xq_ps)

                    # k,v token-major; each tile's kv padded to one PSUM bank
                    kv_ps = ps_kv.tile([128, 2, 512], f32, tag="kv")
                    for i in (0, 1):
                        nc.tensor.matmul(kv_ps[:, i, 0:2 * C],
                                         xT2_sb[:, i, 0:128],
                                         wqkv_bh[:, C:3 * C],
                                         start=True, stop=False)
                        nc.tensor.matmul(kv_ps[:, i, 0:2 * C],
                                         xT2_sb[0:64, i, 128:256],
                                         wqkv_bl[:, C:3 * C],
                                         start=False, stop=True)
                    nc.vector.tensor_copy(kv_sb[:, t0:t0 + 2, :],
                                          kv_ps[:, :, 0:2 * C])
                    # window 0 already sits at partitions 0:64 of kv_sb, so
                    # only window 1 (partitions 64:128) needs the repack DMA
                    sl = slice(t0, t0 + 2)
                    nc.sync.dma_start(out=kv2_sb[:, sl, :],
                                      in_=kv_sb[64:128, sl, :])

                    # qT channel-major [j, tok] + ones column per window
                    qT_ps = ps_small.tile([128, 2, 256], f32, tag="small")
                    for i in (0, 1):
                        xT_hi = xT2_sb[:, i, 0:128]
                        xT_lo = xT2_sb[0:64, i, 128:256]
                        nc.tensor.matmul(qT_ps[:, i, 0:128],
                                         wqkv_bh[:, 0:128], xT_hi,
                                         start=True, stop=False)
                        nc.tensor.matmul(qT_ps[:, i, 0:128],
                                         wqkv_bl[:, 0:128], xT_lo,
                                         start=False, stop=True)
                        nc.tensor.matmul(qT_ps[0:64, i, 128:256],
                                         wqkv_bh[:, 128:192], xT_hi,
                                         start=True, stop=False)
                        nc.tensor.matmul(qT_ps[0:64, i, 128:256],
                                         wqkv_bl[:, 128:192], xT_lo,
                                         start=False, stop=True)
                    # [:, i, g, 0:64] = tokens, [:, i, g, 64] = 1.0;
                    # g in {0,1}: d_hi windows, {2,3}: d_lo windows
                    qT2_sb = sb.tile([128, 2, 4, WIN + 1], bf16, tag="qT")
                    nc.scalar.copy(
                        qT2_sb[:, :, :, 0:WIN],
                        qT_ps.rearrange("p i (g n) -> p i g n", g=4))
                    nc.gpsimd.memset(qT2_sb[:, :, :, WIN:WIN + 1], 1.0)
                    return qT2_sb

                def stage_b_pair(t0, qT2_sb, kv_sb, kv2_sb, fin_sb):
                    # one 2-bank PSUM tile for the pair: tile i at float
                    # offset 512*i (bank-aligned); cols 0:260 = out+D groups,
                    # 260:452 = proj output
                    oy_ps = ps_oy.tile([128, 2, 512], f32, tag="oy")
                    for i in (0, 1):
                        stage_b_half(t0 + i, qT2_sb[:, i, :, :], kv_sb,
                                     kv2_sb,
                                     oy_ps[:, i, 0:260].rearrange(
                                         "p (g n) -> p g n", g=4),
                                     oy_ps[:, i, 260:452])
                    # pair-merged 1/D, divide+cast, and bias-add: one DVE
                    # instruction each (c_lo junk rows are never read)
                    rD = sb.tile([128, 2, 4, 1], f32, tag="rD")
                    for i in (0, 1):
                        nc.vector.reciprocal_approx_fast(
                            out=rD[:, i, :, :],
                            in_=oy_ps[:, i, 0:260].rearrange(
                                "p (g n) -> p g n", g=4)[:, :, WIN:WIN + 1])
                    out_b = sb.tile([128, 2, 4, WIN], bf16, tag="ob")
                    nc.vector.tensor_mul(
                        out_b,
                        oy_ps[:, :, 0:260].rearrange(
                            "p i (g n) -> p i g n", g=4)[:, :, :, 0:WIN],
                        rD.broadcast_to([128, 2, 4, WIN]))
                    for i in (0, 1):
                        nc.tensor.matmul(oy_ps[:, i, 260:452],
                                         out_b[:, i, 0:2, :], wp_bh,
                                         start=True, stop=False)
                        nc.tensor.matmul(oy_ps[:, i, 260:452],
                                         out_b[0:64, i, 2:4, :], wp_bl,
                                         start=False, stop=True)
                    nc.vector.tensor_add(
                        fin_sb[:, t0:t0 + 2, :], oy_ps[:, :, 260:452],
                        b_sb[:, None, :].broadcast_to([128, 2, C]))

                def stage_b_half(t, qT_sb, kv_sb, kv2_sb, out_ps, y_ps):
                    if os.environ.get("K_ABLATE") == "nob":
                        nc.vector.memset(fin_sb[:, t, :], 0.5)
                        return
                    # attnT[d, c] = sum_n v[n, d] k[n, c] per window, into a
                    # 2-bank PSUM tile (window w at float offset 512*w):
                    # hi = [:, w, 0:192], lo = [0:64, w, 192:384]
                    at_ps = ps_attn.tile([128, 2, 512], f32, tag="attn")
                    for w, kvw in ((0, kv_sb[0:64, t, :]),
                                   (1, kv2_sb[:, t, :])):
                        k_ap = kvw[:, 0:C]
                        nc.tensor.matmul(at_ps[:, w, 0:C],
                                         kvw[:, C:C + 128], k_ap,
                                         start=True, stop=True)
                        nc.tensor.matmul(at_ps[0:64, w, C:2 * C],
                                         kvw[:, C + 128:2 * C], k_ap,
                                         start=True, stop=True)

                    # one exp for both windows and both c-halves (rows 64:128
                    # of each lo half are junk and never read)
                    E_sb = sb.tile([128, 2, 2 * C], bf16, tag="E")
                    nc.scalar.activation(E_sb, at_ps[:, :, 0:2 * C], AF.Exp,
                                         scale=SCALE)
                    Ev = E_sb.rearrange("p w (h c) -> p w h c", h=2)
                    Eh_sb = Ev[:, :, 0, :]
                    El_sb = Ev[:, :, 1, :]

                    # out_ps[c, 0:64] = sum_d E[d, c] q[n, d]; [c, 64] = D[c]
                    # groups 0,1 = c_hi windows; 2,3 = c_lo windows
                    for w in range(2):
                        nc.tensor.matmul(out_ps[:, w, :], Eh_sb[:, w, 0:128],
                                         qT_sb[:, w, :], start=True, stop=False)
                        nc.tensor.matmul(out_ps[:, w, :], El_sb[0:64, w, 0:128],
                                         qT_sb[0:64, 2 + w, :],
                                         start=False, stop=True)
                        nc.tensor.matmul(out_ps[0:64, 2 + w, :],
                                         Eh_sb[:, w, 128:192],
                                         qT_sb[:, w, :], start=True, stop=False)
                        nc.tensor.matmul(out_ps[0:64, 2 + w, :],
                                         El_sb[0:64, w, 128:192],
                                         qT_sb[0:64, 2 + w, :],
                                         start=False, stop=True)


                for ci in range(n_chunks):
                    row0 = ci * CH * TOK_TILE
                    x_chunk = x_d[row0:row0 + CH * TOK_TILE, :].rearrange(
                        "(t p) c -> p t c", p=TOK_TILE)
                    y_chunk = y_d[row0:row0 + CH * TOK_TILE, :].rearrange(
                        "(t p) c -> p t c", p=TOK_TILE)

                    x_sb = ck.tile([128, CH, C], f32, tag="x_sb")
                    nc.sync.dma_start(out=x_sb, in_=x_chunk)
                    kv_sb = ck.tile([128, CH, 2 * C], bf16, tag="kv_sb")
                    kv2_sb = ck.tile([64, CH, 2 * C], bf16, tag="kv2_sb")
                    if os.environ.get("K_ABLATE") == "norepack":
                        nc.gpsimd.memset(kv2_sb, 0.125)
                    fin_sb = ck.tile([128, CH, C], f32, tag="fin_sb")

                    RB = int(os.environ.get("K_RB", "1"))
                    for g in range(0, CH // 2, RB):
                        qts = []
                        for tp in range(g, g + RB):
                            t0 = 2 * tp
                            qts.append(
                                (t0, stage_a_pair(t0, x_sb, kv_sb, kv2_sb)))
                        for t0, qT2_sb in qts:
                            stage_b_pair(t0, qT2_sb, kv_sb, kv2_sb, fin_sb)

                    # emit the y store one chunk late: by then its data is
                    # complete, so the DMA generation never blocks the SP
                    # queue ahead of the next chunk's repacks
                    pending_store.append((y_chunk, fin_sb))
                    if len(pending_store) > 1:
                        yc, fs = pending_store.pop(0)
                        nc.sync.dma_start(out=yc, in_=fs)
                for yc, fs in pending_store:
                    nc.sync.dma_start(out=yc, in_=fs)

    nc.compile()
    return nc


def _get_nc(length=L, n_cores=N_CORES, reps=1):
    key = (length, n_cores, reps, CH)
    if key not in _CACHE:
        _CACHE[key] = _build(length, n_cores, reps)
    return _CACHE[key]


def kernel(x, w_qkv, w_proj, b_proj, H=None, W=None, **_unused):
    from concourse.bass_utils import run_bass_kernel_spmd

    x = np.asarray(x, dtype=np.float32)
    w_qkv = np.asarray(w_qkv, dtype=np.float32)
    w_proj = np.asarray(w_proj, dtype=np.float32)
    b_proj = np.asarray(b_proj, dtype=np.float32)
    B, length, c = x.shape
    assert B == N_CORES and c == C

    nc = _get_nc(length, N_CORES)
    in_maps = [
        {"x": np.ascontiguousarray(x[b]), "w_qkv": w_qkv, "w_proj": w_proj,
         "b_proj": b_proj}
        for b in range(B)
    ]
    res = run_bass_kernel_spmd(nc, in_maps, list(range(N_CORES)))
    return np.stack([res.results[b]["y"] for b in range(B)], axis=0)


if __name__ == "__main__":
    # mini smoke test: one chunk worth of tokens per core
    length = int(os.environ.get("K_LEN", CH * TOK_TILE))
    rng = np.random.default_rng(0)
    x = rng.standard_normal((N_CORES, length, C), dtype=np.float32)
    w_qkv = (rng.standard_normal((C, 3 * C)) * 0.02).astype(np.float32)
    w_proj = (rng.standard_normal((C, C)) * 0.02).astype(np.float32)
    b_proj = (rng.standard_normal((C,)) * 0.02).astype(np.float32)

    def ref(x):
        qkv = x @ w_qkv  # [B, L, 3C]
        B_, L_, _ = x.shape
        qkv = qkv.reshape(B_, L_ // 64, 64, 3, C)
        q, k, v = qkv[..., 0, :], qkv[..., 1, :] * SCALE, qkv[..., 2, :]
        attn = np.einsum('bwnc,bwnd->bwcd', k, v)
        attn = np.exp(attn - attn.max(-1, keepdims=True))
        attn = attn / attn.sum(-1, keepdims=True)
        out = np.einsum('bwcd,bwnd->bwnc', attn, q).reshape(B_, L_, C)
        return out @ w_proj + b_proj

    expected = ref(x)
    got = kernel(x, w_qkv, w_proj, b_proj)
    err = np.abs(got - expected).max()
    rel = np.linalg.norm(got - expected) / np.linalg.norm(expected)
    print(f"mini test: max abs err {err:.3e}  rel fro {rel:.3e}")
